# revision 50
# baseline (speedup 1.0000x reference)
"""ColBERT loss kernel for Trainium2, SPMD over 8 NeuronCores.

Problem: q [64,64,128], d_pos/d_neg [64,512,128], mask_pos/neg [64,512] ->
scalar CE loss over maxsim scores [64, 128].

Strategy:
- Shard the 128 docs (64 pos + 64 neg) across 8 cores: 16 docs/core
  (8 pos + 8 neg), replicate q.  Cross-entropy needs full rows, so the
  [64, 16] per-core score slabs are gathered and the tiny [64,128]
  softmax+CE epilogue runs on host.
- Mask folding on host: invalid doc tokens are replaced by that doc's
  token 0 (always valid per setup), so max over the first t_hat
  positions == masked max, exactly (tokens compacted to the front).
- Per core: for each query pair p (two queries -> 128 partitions) and
  2-doc batch: ONE PE matmul sim = qT_p.T @ dT[2 docs] -> PSUM
  [128, 2, t_b] in fp16 (single pass; ~4e-4 loss error).
- The max-over-t reduce runs on the DVE: tensor_reduce(max) straight
  from PSUM over [128, 4 docs, t_b] tiles (1 elem/cycle/lane; this is
  the kernel's roofline).  An optional ACT logsumexp path (X_DVE < 128)
  can offload units to the scalar engine, but ACT's Exp runs at 2
  cyc/elem and its PSUM-tile hold time stalls the 2-tile pipeline, so
  it measures slower and ships disabled.
- Sum over s (64 query tokens) via a ones-matmul on the PE; the tiny
  softmax-CE epilogue runs on host in f64.
"""

import numpy as np

import concourse.bass as bass
import concourse.mybir as mybir
import concourse.tile as tile
from concourse.bass_utils import run_bass_kernel_spmd
from concourse.vector_clock import ScopedClock

B, SQ, SD, H = 64, 64, 512, 128
NCORES = 8
DOCS_PER_CORE = 16  # 8 pos + 8 neg
NPAIRS = B // 2  # 32 query pairs
PAIR_GROUPS = 4  # 8 pairs per sum-matmul group
RED_BATCH = 4  # docs per reduce batch

# Of the NPAIRS*n_batches (pair, doc-batch) units, X_DVE reduce exactly on
# the DVE from PSUM; the rest accumulate exp(p*(sim-C)) on the ACT engine
# (logsumexp ~ max, within ~2e-4 at p=2.2).  Per-batch interleave keeps both
# engines busy inside the 2-tile PSUM window.  Exp costs 2 cyc/elem on ACT
# and runs twice (lo/hi ranges), so the DVE takes ~80% of the units.
X_DVE = 128  # all units on the DVE: the ACT exp path (2 cyc/elem, dual pass)
# measured slower than its PSUM-pipeline cost; lse machinery kept for tuning
N_LSE = 9  # legacy alias; n_lse>0 enables the lse machinery
DUAL_EXP = True
# Measured ACT limits: Ln is valid for inputs in [2^-66, 2^64] (saturates at
# -45.8 below, catastrophic garbage above); Exp floors at ~2^-66 per element,
# so a "dead" accumulator holds ~t*2^-66 ~ 4e-18, not 0.
# p=2.2: lo exp (C=36) valid for doc-max in [~17.8, 55.9], capped at 54.5;
# hi exp (C=70) trusted for doc-max >= 53.3 (acc_hi >= 1e-16, well above the
# dead-entry value ~52.2); handover window [53.3, 54.5].
P_SCALE = 2.2
C_LO = 36.0
C_HI = 70.0
A_LO_CAP = 54.5
ACC_EPS = 1e-19  # added to acc before Ln so its input stays in-range
A_HI_TRUST = 53.3

_DEBUG_DUMP = False


def _dve_flags(x_dve, n_units):
    """n_units booleans, x_dve of them True, evenly interleaved."""
    return [
        (i * x_dve) // n_units != ((i + 1) * x_dve) // n_units
        for i in range(n_units)
    ]


def _merge_order(dve_pairs, lse_pairs):
    """Proportionally interleave the two pair lists so both reduce engines
    stay busy throughout."""
    na, nb = len(dve_pairs), len(lse_pairs)
    order = []
    ia = ib = 0
    for _ in range(na + nb):
        if ib >= nb or (ia < na and ia * (nb or 1) <= ib * (na or 1)):
            order.append(dve_pairs[ia])
            ia += 1
        else:
            order.append(lse_pairs[ib])
            ib += 1
    assert sorted(order) == sorted(dve_pairs + lse_pairs)
    return order


def _patch_tile_drain():
    """walrus rejects >1 sync-wait on a Drain (CTRL) instruction; the
    TileContext tail drain carries one wait per outstanding semaphore.
    Split them across a chain of single-wait drains."""
    if getattr(tile.TileContext, "_drain_patched", False):
        return

    def _drain_and_barrier(self, tick_clock, wait_clock):
        nc = self.nc
        drain_inst = nc.sync.drain()
        wait_clock.add_sem_waits(
            drain_inst.ins, ScopedClock({None: tick_clock.global_clock})
        )
        si = drain_inst.ins.sync_info
        conds = list(si.on_wait) if (si is not None and si.on_wait) else []
        if len(conds) > 1:
            upd = list(si.on_update) if (si is not None and si.on_update) else []
            drain_inst.ins.sync_info = mybir.SyncInfo(on_wait=conds[:1], on_update=upd)
            for c in conds[1:]:
                extra = nc.sync.drain()
                extra.ins.sync_info = mybir.SyncInfo(on_wait=[c], on_update=[])
        nc.all_engine_barrier()
        assert self.sems is not None
        popped = nc._tile_sem_poison_stack.pop()
        assert popped is self._sem_poison
        nc.clear_and_free_semaphores(list(self.sems.allocated().values()))
        nc.all_engine_barrier()

    tile.TileContext._drain_and_barrier = _drain_and_barrier
    tile.TileContext._drain_patched = True


def _split_multi_waits(nc, max_waits=1):
    """This walrus build accepts at most one sync-wait per instruction.
    Hoist extra waits onto same-engine NoOps inserted just before."""
    for f in nc.m.functions:
        for blk in f.blocks:
            new = []
            changed = False
            for inst in blk.instructions:
                si = inst.sync_info
                conds = list(si.on_wait) if (si is not None and si.on_wait) else []
                if len(conds) > max_waits:
                    upd = list(si.on_update) if si.on_update else []
                    for c in conds[:-max_waits]:
                        nop = mybir.InstNoOp(name=f"I-wsplit-{nc.next_id()}")
                        nop.engine = inst.engine
                        nop.sync_info = mybir.SyncInfo(on_wait=[c], on_update=[])
                        new.append(nop)
                    inst.sync_info = mybir.SyncInfo(
                        on_wait=conds[-max_waits:], on_update=upd
                    )
                    changed = True
                new.append(inst)
            if changed:
                blk.instructions = new
    return nc


def _build_program(
    loop_repeat=1,
    probe=None,
    t_hat=SD,
    n_passes=1,
    red_batch=RED_BATCH,
    x_dve=X_DVE,
    dual_exp=DUAL_EXP,
    mm_bufs=2,
):
    """loop_repeat>1 wraps the compute body in a device-side For loop --
    used only for benchmarking.  probe: None | "nored" (skip reduces).
    t_hat: per-batch token counts (compacted inputs put valid tokens
    first).  x_dve: units reduced directly on the DVE (rest via lse)."""
    _patch_tile_drain()
    f32 = mybir.dt.float32
    f16 = mybir.dt.float16
    nc = bass.Bass("TRN2", target_bir_lowering=False, debug=False, num_devices=NCORES)

    qh = nc.dram_tensor("qh", [H, B * SQ], f16, kind="ExternalInput").ap()
    dh = nc.dram_tensor("dh", [H, DOCS_PER_CORE * SD], f16, kind="ExternalInput").ap()
    if n_passes >= 2:
        ql = nc.dram_tensor("ql", [H, B * SQ], f16, kind="ExternalInput").ap()
    ones2 = nc.dram_tensor("ones2", [H, 2], f16, kind="ExternalInput").ap()
    out = nc.dram_tensor(
        "scores_raw", [H, PAIR_GROUPS * 2], f32, kind="ExternalOutput"
    ).ap()
    dbg = nc.dram_tensor(
        "dbg", [H, NPAIRS * DOCS_PER_CORE], f16, kind="ExternalOutput"
    ).ap() if _DEBUG_DUMP else None
    dbg2 = nc.dram_tensor(
        "dbg2", [H, 2 * NPAIRS * DOCS_PER_CORE], f32, kind="ExternalOutput"
    ).ap() if (_DEBUG_DUMP and x_dve < NPAIRS * (DOCS_PER_CORE // red_batch)) else None

    import contextlib

    with tile.TileContext(nc) as tc, contextlib.ExitStack() as es:
        const_pool = es.enter_context(tc.tile_pool(name="const", bufs=1))
        mm_pool = es.enter_context(tc.tile_pool(name="mm", bufs=mm_bufs, space="PSUM"))
        sum_pool = mm_pool  # sum-matmul tiles recycle the mm slots
        sb_pool = es.enter_context(tc.tile_pool(name="sb", bufs=1))

        qh_sb = const_pool.tile([H, B * SQ], f16)
        nc.sync.dma_start(qh_sb[:], qh[:])
        ql_sb = None
        if n_passes >= 2:
            ql_sb = const_pool.tile([H, B * SQ], f16)
            nc.sync.dma_start(ql_sb[:], ql[:])
        dh_sb = const_pool.tile([H, DOCS_PER_CORE, SD], f16)
        nc.sync.dma_start(dh_sb[:], dh[:])
        ones2_sb = const_pool.tile([H, 2], f16)
        nc.sync.dma_start(ones2_sb[:], ones2[:])

        # each doc row occupies exactly one PSUM bank so per-doc matmul
        # outputs never cross bank boundaries (corruption otherwise)
        t_cap = SD

        maxvals = sb_pool.tile([H, NPAIRS, DOCS_PER_CORE], f16)
        scores_sb = sb_pool.tile([H, PAIR_GROUPS, 2], f32)
        nc.vector.memset(maxvals[:], 0.0)
        nc.vector.memset(scores_sb[:], 0.0)

        n_units = NPAIRS * (DOCS_PER_CORE // red_batch)
        lse = None
        if x_dve < n_units:
            acc_lo = sb_pool.tile([H, NPAIRS, DOCS_PER_CORE], f32)
            zbuf = sb_pool.tile([H, NPAIRS, DOCS_PER_CORE], f32)
            a_lo = sb_pool.tile([H, NPAIRS, DOCS_PER_CORE], f16)
            exp_trash = sb_pool.tile([H, SD], f32)
            bias_lo = sb_pool.tile([H, 1], f32)
            eps_bias = sb_pool.tile([H, 1], f32)
            lse = {
                "acc_lo": acc_lo,
                "zbuf": zbuf,
                "a_lo": a_lo,
                "trash": exp_trash,
                "bias_lo": bias_lo,
                "eps": eps_bias,
            }
            nc.vector.memset(bias_lo[:], -P_SCALE * C_LO)
            nc.vector.memset(acc_lo[:], 0.0)
            nc.vector.memset(eps_bias[:], ACC_EPS)
            if dual_exp:
                acc_hi = sb_pool.tile([H, NPAIRS, DOCS_PER_CORE], f32)
                a_hi = sb_pool.tile([H, NPAIRS, DOCS_PER_CORE], f16)
                bias_hi = sb_pool.tile([H, 1], f32)
                lse["acc_hi"] = acc_hi
                lse["a_hi"] = a_hi
                lse["bias_hi"] = bias_hi
                nc.vector.memset(bias_hi[:], -P_SCALE * C_HI)
                nc.vector.memset(acc_hi[:], 0.0)

        def body(_iv=None):
            _emit_body(
                nc,
                qh_sb,
                ql_sb,
                dh_sb,
                ones2_sb,
                maxvals,
                scores_sb,
                mm_pool,
                sum_pool,
                lse,
                probe,
                t_hat,
                t_cap,
                n_passes,
                red_batch,
                x_dve,
                dual_exp,
            )

        if loop_repeat > 1:
            with tc.For_i(0, loop_repeat, 1):
                body()
        else:
            body()

        nc.sync.dma_start(out[:], scores_sb[:, :, :])
        if dbg is not None:
            nc.sync.dma_start(dbg[:], maxvals[:])
        if dbg2 is not None:
            nc.sync.dma_start(dbg2[:, 0 : n_lse * DOCS_PER_CORE], lse["acc_lo"][:])
            nc.sync.dma_start(
                dbg2[:, n_lse * DOCS_PER_CORE : 2 * n_lse * DOCS_PER_CORE],
                lse["acc_hi"][:],
            )

    _split_multi_waits(nc)
    return nc


def _emit_body(
    nc,
    qh_sb,
    ql_sb,
    dh_sb,
    ones2_sb,
    maxvals,
    scores_sb,
    mm_pool,
    sum_pool,
    lse,
    probe,
    t_hat,
    t_cap,
    n_passes,
    red_batch,
    x_dve,
    dual_exp,
):
    f32 = mybir.dt.float32
    f16 = mybir.dt.float16
    n_batches = DOCS_PER_CORE // red_batch
    batch_ts = [t_hat] * n_batches if isinstance(t_hat, int) else list(t_hat)
    assert len(batch_ts) == n_batches
    flags = _dve_flags(x_dve, NPAIRS * n_batches)

    for p in range(NPAIRS):
        qslice = slice(p * 128, (p + 1) * 128)
        for b in range(n_batches):
            use_lse = not flags[p * n_batches + b]
            t_b = batch_ts[b]
            ps = mm_pool.tile([H, red_batch, t_cap], f32, tag="ps")
            if n_passes == 1:
                passes = [(qh_sb, True, True)]
            else:
                passes = [(qh_sb, True, False), (ql_sb, False, True)]
            for w_sb, is_start, is_stop in passes:
                for j in range(red_batch):
                    nc.tensor.matmul(
                        ps[:, j, 0:t_b],
                        lhsT=w_sb[:, qslice],
                        rhs=dh_sb[:, b * red_batch + j, 0:t_b],
                        start=is_start,
                        stop=is_stop,
                    )
            if probe == "nored":
                continue
            if use_lse:
                for j in range(red_batch):
                    d = b * red_batch + j
                    nc.scalar.activation(
                        lse["trash"][:, 0:t_b],
                        ps[:, j, 0:t_b],
                        mybir.ActivationFunctionType.Exp,
                        bias=lse["bias_lo"][:],
                        scale=P_SCALE,
                        accum_out=lse["acc_lo"][:, p, d : d + 1],
                    )
                    if dual_exp:
                        nc.scalar.activation(
                            lse["trash"][:, 0:t_b],
                            ps[:, j, 0:t_b],
                            mybir.ActivationFunctionType.Exp,
                            bias=lse["bias_hi"][:],
                            scale=P_SCALE,
                            accum_out=lse["acc_hi"][:, p, d : d + 1],
                        )
            else:
                nc.vector.tensor_reduce(
                    out=maxvals[:, p, b * red_batch : (b + 1) * red_batch],
                    in_=ps[:, :, 0:t_b],
                    axis=mybir.AxisListType.X,
                    op=mybir.AluOpType.max,
                )

    if probe == "nored":
        return
    if lse is not None:
        # maxvals[lse pairs] = ln(acc)/p + C, folded across the lo/hi exps
        # (the out-of-range exp underflows to ln(0) = -inf and loses the max;
        # the lo exp can overflow to +inf, so cap it below the hi range).
        # acc += eps on the ACT queue (in-order after the exps -- accum_out
        # writes are not tracked by Tile, so DVE-side clamps would race),
        # keeping the Ln input inside its valid range (>= ~2^-87).
        inv_p = 1.0 / P_SCALE
        nc.scalar.activation(
            lse["acc_lo"][:],
            lse["acc_lo"][:],
            mybir.ActivationFunctionType.Identity,
            bias=lse["eps"][:],
        )
        nc.scalar.activation(
            lse["zbuf"][:],
            lse["acc_lo"][:],
            mybir.ActivationFunctionType.Ln,
        )
        nc.vector.tensor_scalar(
            out=lse["a_lo"][:],
            in0=lse["zbuf"][:],
            scalar1=inv_p,
            scalar2=C_LO,
            op0=mybir.AluOpType.mult,
            op1=mybir.AluOpType.add,
        )
        if dual_exp:
            nc.vector.tensor_scalar_min(lse["a_lo"][:], lse["a_lo"][:], A_LO_CAP)
            nc.scalar.activation(
                lse["acc_hi"][:],
                lse["acc_hi"][:],
                mybir.ActivationFunctionType.Identity,
                bias=lse["eps"][:],
            )
            nc.scalar.activation(
                lse["zbuf"][:],
                lse["acc_hi"][:],
                mybir.ActivationFunctionType.Ln,
            )
            nc.vector.tensor_scalar(
                out=lse["a_hi"][:],
                in0=lse["zbuf"][:],
                scalar1=inv_p,
                scalar2=C_HI,
                op0=mybir.AluOpType.mult,
                op1=mybir.AluOpType.add,
            )
            # only trust a_hi above A_HI_TRUST; zeroed entries lose the max
            nc.vector.tensor_scalar(
                out=lse["zbuf"][:],
                in0=lse["a_hi"][:],
                scalar1=A_HI_TRUST,
                scalar2=None,
                op0=mybir.AluOpType.is_gt,
            )
            nc.vector.tensor_tensor(
                lse["a_hi"][:],
                lse["a_hi"][:],
                lse["zbuf"][:],
                mybir.AluOpType.mult,
            )
            nc.vector.tensor_tensor(
                lse["a_lo"][:],
                lse["a_lo"][:],
                lse["a_hi"][:],
                mybir.AluOpType.max,
            )
        # combine with the DVE-direct entries (lse entries hold 0 there,
        # and every valid lse value exceeds the a_lo floor ~17.8 > 0)
        nc.vector.tensor_tensor(
            maxvals[:],
            maxvals[:],
            lse["a_lo"][:],
            mybir.AluOpType.max,
        )
    for g in range(PAIR_GROUPS):
        sums = sum_pool.tile([H, 2], f32, tag="ps")
        nc.tensor.matmul(
            sums[:],
            lhsT=maxvals[:, g * 8 : (g + 1) * 8, :],
            rhs=ones2_sb[:],
            start=True,
            stop=True,
        )
        nc.vector.tensor_copy(scores_sb[:, g, :], sums[:])


_PROGRAMS = {}

N_PASSES = 1


def _get_program(batch_ts):
    key = (tuple(batch_ts), N_PASSES, X_DVE, DUAL_EXP)
    if key not in _PROGRAMS:
        _PROGRAMS[key] = _build_program(t_hat=tuple(batch_ts), n_passes=N_PASSES)
    return _PROGRAMS[key]


def _host_prep(q, d_pos, d_neg, mask_pos, mask_neg):
    q = np.asarray(q, dtype=np.float32)
    d_pos = np.asarray(d_pos, dtype=np.float32)
    d_neg = np.asarray(d_neg, dtype=np.float32)
    mask_pos = np.asarray(mask_pos)
    mask_neg = np.asarray(mask_neg)

    # Compact: move each doc's valid tokens to the front, pad the tail
    # with copies of token 0 (always valid per setup).  Plain max over
    # the first t_hat columns == masked max, exactly.
    def compact(d, mask):
        out = np.empty_like(d)
        for b in range(d.shape[0]):
            v = d[b, mask[b] != 0]
            out[b, : len(v)] = v
            out[b, len(v) :] = d[b, 0]
        return out

    dp = compact(d_pos, mask_pos)
    dn = compact(d_neg, mask_neg)
    cp = mask_pos.sum(1)
    cn = mask_neg.sum(1)
    # Global doc-to-core assignment: sort all 128 docs by valid-count
    # descending and deal round-robin, so core c slot k holds global rank
    # 8k+c and batch b's padded T is exactly the rank-32b count (the
    # tightest possible uniform batch width).
    counts_all = np.concatenate([cp, cn])  # global id: 0-63 pos, 64-127 neg
    order = np.argsort(-counts_all, kind="stable")
    assign = [
        order[np.arange(DOCS_PER_CORE) * NCORES + c] for c in range(NCORES)
    ]
    n_batches = DOCS_PER_CORE // RED_BATCH
    sorted_counts_g = counts_all[order]
    batch_ts = tuple(
        min(SD, int(sorted_counts_g[b * RED_BATCH * NCORES]))
        for b in range(n_batches)
    )

    def split_hi_lo(x):
        hi = x.astype(np.float16)
        lo = (x - hi.astype(np.float32)).astype(np.float16)
        return hi, lo

    # qT[h, q*SQ + s]
    qT = np.ascontiguousarray(q.transpose(2, 0, 1).reshape(H, B * SQ))
    qh, ql = split_hi_lo(qT)
    # dT[h, doc, t]
    dpT = dp.transpose(2, 0, 1)  # [H, 64, 512]
    dnT = dn.transpose(2, 0, 1)

    ones2 = np.zeros((H, 2), np.float16)
    ones2[:SQ, 0] = 1.0
    ones2[SQ:, 1] = 1.0

    dT_all = np.concatenate([dpT, dnT], axis=1)  # [H, 128, 512]
    in_maps = []
    for c in range(NCORES):
        dT_c = np.ascontiguousarray(
            dT_all[:, assign[c], :].reshape(H, DOCS_PER_CORE * SD)
        )
        dh_c = dT_c.astype(np.float16)
        in_maps.append(
            {
                "qh": qh,
                "ql": ql,
                "dh": dh_c,
                "ones2": ones2,
            }
        )
    return in_maps, batch_ts, assign


def _host_epilogue(results, perms):
    # scores_raw rows: partition = pg*16 + d_local; cols: g*2 + j
    # query = 2*(8*g + pg) + j ; doc_local d: 0-7 pos docs 8c+d, 8-15 neg.
    dist = np.zeros((B, 2 * B), np.float32)
    for c in range(NCORES):
        arr = np.asarray(results[c]["scores_raw"])  # [128, 8]
        arr = arr.reshape(8, 16, PAIR_GROUPS, 2)  # [pg, d, g, j]
        s_qd = arr.transpose(2, 0, 3, 1).reshape(B, DOCS_PER_CORE)  # [query, slot]
        # slot k holds global doc id perms[c][k]; global id == dist column
        # (pos docs 0-63 -> cols 0-63, neg docs 64-127 -> cols 64-127)
        dist[:, perms[c]] = s_qd

    d64 = dist.astype(np.float64)
    m = d64.max(axis=1, keepdims=True)
    logz = np.log(np.exp(d64 - m).sum(axis=1)) + m[:, 0]
    lbl = np.arange(B)
    loss = -(d64[lbl, lbl] - logz).mean()
    return np.array(loss, dtype=np.float32)


def kernel(q, d_pos, d_neg, mask_pos, mask_neg):
    in_maps, batch_ts, perms = _host_prep(q, d_pos, d_neg, mask_pos, mask_neg)
    nc = _get_program(batch_ts)
    res = run_bass_kernel_spmd(nc, in_maps, list(range(NCORES)), trace=False)
    return _host_epilogue(res.results, perms)
